# revision 1
# baseline (speedup 1.0000x reference)
"""FSUMGU cell on 8 Trainium2 NeuronCores.

Math (per reference):
    zf = [hx, x] @ w_f.T + b_f
    fg = (zf + 1) / 2
    fgx = fg * hx
    ng = [fgx, x] @ w_n.T + b_n
    hy = (1 - fg) * ng + fgx

Sharding: 2 batch-halves (r) x 4 hidden-quarters (c); core id = r*4 + c.
Each core computes hy[r-half, c-quarter]. The only cross-core dependency
is ng's contraction over the full hidden dim of fgx, satisfied with one
AllGather of bf16 fgx^T over each 4-core row group.

On-core: activations/weights are PE-transposed into [k, *] bf16 SBUF
tiles so every matmul is out[b,h] += catT[k,b].T @ wT[k,h] with fp32
PSUM accumulation. fp32->bf16 happens inside SWDGE cast-DMAs. PE
transposes are interleaved into the matmul stream in small groups so
the tensor engine never idles long enough for HAM to re-throttle the
clock. Phase 2 accumulates its input-half contraction first so those
matmuls (plus the w_n transposes) hide the AllGather latency.
"""
import sys

sys.path.insert(0, "/opt/trn_rl_repo")

import numpy as np
import concourse.bass as bass
import concourse.tile as tile
from concourse import bacc, mybir, masks
from concourse.bass_utils import run_bass_kernel_spmd

F32 = mybir.dt.float32
BF16 = mybir.dt.bfloat16

B, H, I = 2048, 2048, 2048
R, C = 2, 4
BL = B // R            # 1024 rows of batch per core
HC = H // C            # 512 output features per core
NB = BL // 128         # 8 batch tiles
NKH = H // 128         # 16 k-tiles in the hx / fgx part
NKI = I // 128         # 16 k-tiles in the input part
NK = NKH + NKI         # 32 k-tiles total contraction
NHT = HC // 128        # 4 h-tiles per core slice

_NC_CACHE = None


def build():
    nc = bacc.Bacc(None, target_bir_lowering=False, debug=False)
    d_inp = nc.dram_tensor("inp", [BL, I], F32, kind="ExternalInput").ap()
    d_hx = nc.dram_tensor("hx", [BL, H], F32, kind="ExternalInput").ap()
    d_hxc = nc.dram_tensor("hxc", [BL, HC], F32, kind="ExternalInput").ap()
    d_wf = nc.dram_tensor("wf", [HC, H + I], F32, kind="ExternalInput").ap()
    d_wn = nc.dram_tensor("wn", [HC, H + I], F32, kind="ExternalInput").ap()
    d_bf = nc.dram_tensor("bf", [1, HC], F32, kind="ExternalInput").ap()
    d_bn = nc.dram_tensor("bn", [1, HC], F32, kind="ExternalInput").ap()
    d_hy = nc.dram_tensor("hy", [BL, HC], F32, kind="ExternalOutput").ap()

    with tile.TileContext(nc) as tc:
        with (
            tc.tile_pool(name="const", bufs=1) as const,
            tc.tile_pool(name="wT", bufs=1) as wT_pool,          # wfT then wnT (time-shared)
            tc.tile_pool(name="big", bufs=1) as big_pool,        # hxT then gather (time-shared) + inputT
            tc.tile_pool(name="persist", bufs=1) as persist,
            tc.tile_pool(name="aload", bufs=4) as aload,
            tc.tile_pool(name="wload", bufs=2) as wload,
            tc.tile_pool(name="wnload", bufs=2) as wnload,
            tc.tile_pool(name="scr", bufs=2) as scr,
            tc.tile_pool(name="fgtr", bufs=1) as fgtr,
            tc.tile_pool(name="outp", bufs=2) as outp,
            tc.tile_pool(name="dram", bufs=1, space="DRAM") as dram,
            tc.tile_pool(name="ps_acc", bufs=5, space="PSUM") as ps_acc,
            tc.tile_pool(name="ps_tp", bufs=3, space="PSUM") as ps_tp,
        ):
            ident = const.tile([128, 128], BF16, tag="ident")
            masks.make_identity(nc, ident[:])
            ones = const.tile([1, 128], BF16, tag="ones")
            nc.vector.memset(ones[:], 1.0)

            # ---- persistent transposed tensors
            hxT = big_pool.tile([128, NKH, BL], BF16, tag="big_hx")      # hx^T  (phase 1)
            inputT = big_pool.tile([128, NKI, BL], BF16, tag="big_inp")  # input^T (both phases)
            wfT = wT_pool.tile([128, NK, HC], BF16, tag="wT")
            fg_hxT = persist.tile([128, NHT, BL], BF16, tag="fghxT")
            omfgN = persist.tile([128, NB, HC], BF16, tag="omfg")
            fghxN = persist.tile([128, NB, HC], BF16, tag="fghx")
            hxcN = persist.tile([128, NB, HC], BF16, tag="hxc")
            spillN = persist.tile([128, 5, HC], BF16, tag="spill")

            # DRAM bounce buffers for the two collectives (split 3/5 so the
            # first gather can trigger early, mid phase 1)
            NB1 = 3
            HB = NB1 * 128
            HB2 = BL - HB
            cc_in1 = dram.tile([HC, HB], BF16)
            cc_in2 = dram.tile([HC, HB2], BF16)
            cc_out1 = dram.tile([C, HC, HB], BF16)
            cc_out2 = dram.tile([C, HC, HB2], BF16)

            eng_state = [0]

            def emit_tp_group(src_b16, src_k0, n_kt, dst, dst_ti0, dst_col):
                """PE-transpose n_kt (<=4) [128,128] slices + one batched copy."""
                tp = ps_tp.tile([128, 512], BF16, tag="tp")
                for j in range(n_kt):
                    nc.tensor.matmul(
                        tp[:, j * 128:(j + 1) * 128],
                        src_b16[:, (src_k0 + j) * 128:(src_k0 + j + 1) * 128],
                        ident[:],
                        is_transpose=True,
                    )
                dst_ap = dst[:, dst_ti0:dst_ti0 + n_kt, dst_col:dst_col + 128]
                src_ap = tp[:, :n_kt * 128].rearrange("p (a f) -> p a f", f=128)
                if eng_state[0] % 2 == 0:
                    nc.vector.tensor_copy(dst_ap, src_ap)
                else:
                    nc.scalar.copy(dst_ap, src_ap)
                eng_state[0] += 1

            filler = []   # queued (tag, fn) transpose groups, drained between MM bursts

            def drain(n):
                for _ in range(min(n, len(filler))):
                    filler.pop(0)[1]()

            def drain_until(tag):
                """Emit every queued group with tag <= `tag` (correctness gate)."""
                while filler and filler[0][0] <= tag:
                    filler.pop(0)[1]()

            def queue_act_tiles(bt):
                """Load + queue transposes for b-tile bt's activations."""
                bs = bt * 128
                achunk = aload.tile([128, NKH * 128], BF16, tag="aload")
                nc.gpsimd.dma_start(achunk[:], d_hx[bs:bs + 128, :])
                ichunk = aload.tile([128, NKI * 128], BF16, tag="aload")
                nc.gpsimd.dma_start(ichunk[:], d_inp[bs:bs + 128, :])
                nc.gpsimd.dma_start(hxcN[:, bt, :], d_hxc[bs:bs + 128, :])
                for g in range(0, NKH, 4):
                    filler.append((bt, lambda g=g, t=achunk: emit_tp_group(t, g, 4, hxT, g, bs)))
                for g in range(0, NKI, 4):
                    filler.append((bt, lambda g=g, t=ichunk: emit_tp_group(t, g, 4, inputT, g, bs)))

            # ---- w_f k-half 0 + first activations: minimal deps for first matmul
            def load_wf_half(kh, dst):
                for a in range(NHT):
                    wchunk = wload.tile([128, NKH * 128], BF16, tag="wload")
                    nc.gpsimd.dma_start(
                        wchunk[:], d_wf[a * 128:(a + 1) * 128, kh * 2048:(kh + 1) * 2048])
                    for g in range(0, NKH, 4):
                        emit_tp_group(wchunk, g, 4, dst, kh * NKH + g, a * 128)

            load_wf_half(0, wfT)
            queue_act_tiles(0)
            queue_act_tiles(1)
            drain_until(1)

            # ---- bias prep: bfp=(b_f+1)/2, bfm=1-bfp, bn; broadcast to 128 partitions
            bf_row = const.tile([1, HC], F32, tag="bfrow")
            bn_row = const.tile([1, HC], F32, tag="bnrow")
            nc.sync.dma_start(bf_row[:], d_bf[:])
            nc.sync.dma_start(bn_row[:], d_bn[:])
            bfp_row = const.tile([1, HC], F32, tag="bfprow")
            bfm_row = const.tile([1, HC], F32, tag="bfmrow")
            nc.vector.tensor_scalar(bfp_row[:], bf_row[:], 0.5, 0.5,
                                    mybir.AluOpType.mult, mybir.AluOpType.add)
            nc.vector.tensor_scalar(bfm_row[:], bfp_row[:], -1.0, 1.0,
                                    mybir.AluOpType.mult, mybir.AluOpType.add)
            bias_bc = const.tile([128, 3, HC], BF16, tag="biasbc")
            for bi, row in enumerate((bfp_row, bfm_row, bn_row)):
                row16 = const.tile([1, HC], BF16, tag=f"row16_{bi}")
                nc.vector.tensor_copy(row16[:], row[:])
                pb = ps_tp.tile([128, HC], F32, tag="tp")
                nc.tensor.matmul(pb[:], ones[:], row16[:], start=True, stop=True)
                nc.vector.tensor_copy(bias_bc[:, bi, :], pb[:])
            bfp_bc = bias_bc[:, 0, :]
            bfm_bc = bias_bc[:, 1, :]
            bn_bc = bias_bc[:, 2, :]

            wn_anchor = [None]

            # ---- phase 1 per batch tile: dense MM stream + interleaved fillers
            for bt in range(NB):
                bs = bt * 128
                if bt + 2 < NB:
                    queue_act_tiles(bt + 2)
                drain_until(bt)  # this tile's operands must be emitted already
                acc = ps_acc.tile([128, HC], F32, tag="acc")
                for j in range(NK):
                    # interleave queued transposes only once DMA runs ahead of
                    # the PE; early on a not-yet-loaded group would stall the
                    # statically-ordered matmul stream.
                    if bt >= 3 and j % 8 == 4:
                        drain(1)
                    if bt == 0 and j == NKH:
                        # w_f k-half 1 transposes slot in after bt0's first
                        # half-contraction, hiding their DMA behind real work
                        load_wf_half(1, wfT)
                    lhsT = (hxT[:, j, bs:bs + 128] if j < NKH
                            else inputT[:, j - NKH, bs:bs + 128])
                    nc.tensor.matmul(
                        acc[:], lhsT, wfT[:, j, :],
                        start=(j == 0), stop=(j == NK - 1),
                    )
                # fg = 0.5*acc + bfp ; omfg = bfm - 0.5*acc ; fgx = fg * hxc
                fg_t = fgtr.tile([128, HC], BF16, tag="fg")
                fg_inst = nc.vector.scalar_tensor_tensor(
                    fg_t[:], acc[:], 0.5, bfp_bc,
                    mybir.AluOpType.mult, mybir.AluOpType.add)
                if bt == 2:
                    wn_anchor[0] = fg_inst
                nc.vector.scalar_tensor_tensor(
                    omfgN[:, bt, :], acc[:], -0.5, bfm_bc,
                    mybir.AluOpType.mult, mybir.AluOpType.add)
                nc.vector.tensor_mul(fghxN[:, bt, :], fg_t[:], hxcN[:, bt, :])
                # transpose fgx tile -> fg_hxT[:, :, bs:bs+128] (small, HAM-safe)
                tp = ps_tp.tile([128, 512], BF16, tag="tp")
                for a in range(NHT):
                    nc.tensor.matmul(
                        tp[:, a * 128:(a + 1) * 128],
                        fghxN[:, bt, a * 128:(a + 1) * 128],
                        ident[:],
                        is_transpose=True,
                    )
                nc.scalar.copy(
                    fg_hxT[:, :, bs:bs + 128],
                    tp[:].rearrange("p (a f) -> p a f", f=128),
                )
                # stream this b-tile's fgx^T columns to the collective input
                cc_in_half = cc_in1 if bt < NB1 else cc_in2
                hb = bs if bt < NB1 else bs - HB
                nc.sync.dma_start(
                    cc_in_half.rearrange("(a p) b -> p a b", p=128)[:, :, hb:hb + 128],
                    fg_hxT[:, :, bs:bs + 128])
                if bt == NB1 - 1:
                    # first-half all-gather rides under the rest of phase 1
                    nc.gpsimd.collective_compute(
                        "AllGather",
                        mybir.AluOpType.bypass,
                        replica_groups=[[0, 1, 2, 3], [4, 5, 6, 7]],
                        ins=[cc_in1.opt()],
                        outs=[cc_out1.opt()],
                    )

            # ---- second-half all-gather
            nc.gpsimd.collective_compute(
                "AllGather",
                mybir.AluOpType.bypass,
                replica_groups=[[0, 1, 2, 3], [4, 5, 6, 7]],
                ins=[cc_in2.opt()],
                outs=[cc_out2.opt()],
            )

            # ---- w_n: load input-half (k-tiles 16..31) first, transpose all.
            # This dense block (plus phase-2's input-half matmuls) runs during
            # the AllGather, so PE idle time there is already covered.
            wnT = wT_pool.tile([128, NK, HC], BF16, tag="wT")
            from concourse.tile import add_dep_helper
            for kh in (1, 0):
                for a in range(NHT):
                    wchunk = wnload.tile([128, NKH * 128], BF16, tag="wnload")
                    wdma = nc.gpsimd.dma_start(
                        wchunk[:], d_wn[a * 128:(a + 1) * 128, kh * 2048:(kh + 1) * 2048])
                    # keep w_n traffic out of phase 1's DMA window
                    add_dep_helper(wdma.ins, wn_anchor[0].ins, sync=True,
                                   reason="defer w_n loads past mid phase 1")
                    for g in range(0, NKH, 4):
                        emit_tp_group(wchunk, g, 4, wnT, kh * NKH + g, a * 128)
            drain(len(filler))  # flush any remaining queued act groups
            assert not filler

            # ---- read back gathered fgx^T (reuses hxT's slot)
            gat = big_pool.tile([128, NKH, BL], BF16, tag="big_hx")
            for j in range(NKH):
                q, rr = j // NHT, (j % NHT) * 128
                nc.sync.dma_start(gat[:, j, :HB], cc_out1[q, rr:rr + 128, :])
                nc.sync.dma_start(gat[:, j, HB:], cc_out2[q, rr:rr + 128, :])

            # ---- phase 2. B-tiles 0-4 hold their PSUM bank across both
            # contraction halves; 5-7 run their (CC-independent) input half
            # eagerly and spill the partial to SBUF so those matmuls fit in
            # the second all-gather's window.
            def epilogue(bt, acc, spill=None):
                bs = bt * 128
                t = scr.tile([128, HC], F32, tag="t")
                nc.vector.tensor_add(t[:], acc[:], bn_bc)
                if spill is not None:
                    nc.vector.tensor_add(t[:], t[:], spill)
                u = scr.tile([128, HC], F32, tag="u")
                nc.vector.tensor_mul(u[:], omfgN[:, bt, :], t[:])
                o = outp.tile([128, HC], F32, tag="o")
                nc.vector.tensor_add(o[:], u[:], fghxN[:, bt, :])
                nc.sync.dma_start(d_hy[bs:bs + 128, :], o[:])

            for bt in range(NB1):
                bs = bt * 128
                acc = ps_acc.tile([128, HC], F32, tag="acc")
                korder = list(range(NKH, NK)) + list(range(NKH))
                for idx, j in enumerate(korder):
                    lhsT = (gat[:, j, bs:bs + 128] if j < NKH
                            else inputT[:, j - NKH, bs:bs + 128])
                    nc.tensor.matmul(
                        acc[:], lhsT, wnT[:, j, :],
                        start=(idx == 0), stop=(idx == NK - 1),
                    )
                epilogue(bt, acc)
            for bt in range(NB1, NB):
                bs = bt * 128
                acc = ps_acc.tile([128, HC], F32, tag="acc")
                for idx, j in enumerate(range(NKH, NK)):
                    nc.tensor.matmul(
                        acc[:], inputT[:, j - NKH, bs:bs + 128], wnT[:, j, :],
                        start=(idx == 0), stop=(idx == NKH - 1),
                    )
                nc.vector.tensor_copy(spillN[:, bt - NB1, :], acc[:])
            for bt in range(NB1, NB):
                bs = bt * 128
                acc = ps_acc.tile([128, HC], F32, tag="acc")
                for idx, j in enumerate(range(NKH)):
                    nc.tensor.matmul(
                        acc[:], gat[:, j, bs:bs + 128], wnT[:, j, :],
                        start=(idx == 0), stop=(idx == NKH - 1),
                    )
                epilogue(bt, acc, spill=spillN[:, bt - NB1, :])

    nc.finalize()
    return nc


def _get_nc():
    global _NC_CACHE
    if _NC_CACHE is None:
        _NC_CACHE = build()
    return _NC_CACHE


def kernel(input, hx, w_f, b_f, w_n, b_n, **_ignored):
    input = np.ascontiguousarray(np.asarray(input, dtype=np.float32))
    hx = np.ascontiguousarray(np.asarray(hx, dtype=np.float32))
    w_f = np.ascontiguousarray(np.asarray(w_f, dtype=np.float32))
    b_f = np.ascontiguousarray(np.asarray(b_f, dtype=np.float32))
    w_n = np.ascontiguousarray(np.asarray(w_n, dtype=np.float32))
    b_n = np.ascontiguousarray(np.asarray(b_n, dtype=np.float32))

    nc = _get_nc()
    in_maps = []
    for core in range(R * C):
        r, c = core // C, core % C
        in_maps.append({
            "inp": np.ascontiguousarray(input[r * BL:(r + 1) * BL, :]),
            "hx": np.ascontiguousarray(hx[r * BL:(r + 1) * BL, :]),
            "hxc": np.ascontiguousarray(hx[r * BL:(r + 1) * BL, c * HC:(c + 1) * HC]),
            "wf": np.ascontiguousarray(w_f[c * HC:(c + 1) * HC, :]),
            "wn": np.ascontiguousarray(w_n[c * HC:(c + 1) * HC, :]),
            "bf": np.ascontiguousarray(b_f[None, c * HC:(c + 1) * HC]),
            "bn": np.ascontiguousarray(b_n[None, c * HC:(c + 1) * HC]),
        })
    res = run_bass_kernel_spmd(nc, in_maps, list(range(R * C)))
    rows = []
    for r in range(R):
        rows.append(np.concatenate(
            [res.results[r * C + c]["hy"] for c in range(C)], axis=1))
    return np.concatenate(rows, axis=0)


if __name__ == "__main__":
    rng = np.random.default_rng(0)
    inputs = {
        "input": rng.uniform(-1, 1, (B, I)).astype(np.float32),
        "hx": rng.uniform(-1, 1, (B, H)).astype(np.float32),
        "w_f": (rng.standard_normal((H, H + I)) / np.sqrt(H + I)).astype(np.float32),
        "b_f": (rng.standard_normal(H) / np.sqrt(H + I)).astype(np.float32),
        "w_n": (rng.standard_normal((H, H + I)) / np.sqrt(H + I)).astype(np.float32),
        "b_n": (rng.standard_normal(H) / np.sqrt(H + I)).astype(np.float32),
    }
    out = kernel(**inputs)
    x64 = {k: v.astype(np.float64) for k, v in inputs.items()}
    cat = np.concatenate([x64["hx"], x64["input"]], axis=1)
    fg = (cat @ x64["w_f"].T + x64["b_f"] + 1.0) * 0.5
    fgx = fg * x64["hx"]
    ng = np.concatenate([fgx, x64["input"]], axis=1) @ x64["w_n"].T + x64["b_n"]
    exp = (1.0 - fg) * ng + fgx
    err = np.abs(out - exp).max() / np.abs(exp).max()
    print("rel err:", err)



# revision 3
# speedup vs baseline: 1.3766x; 1.3766x over previous
"""FSUMGU cell on 8 Trainium2 NeuronCores — transposed-space formulation.

Math (per reference):
    zf = [hx, x] @ w_f.T + b_f
    fg = (zf + 1) / 2
    fgx = fg * hx
    ng = [fgx, x] @ w_n.T + b_n
    hy = (1 - fg) * ng + fgx

Sharding: 2 batch-halves (r) x 4 hidden-quarters (c); core id = r*4 + c.

Everything on-core is computed in TRANSPOSED space (hy^T[h, b]), which
makes every matmul operand naturally k-major:
    zf^T[h, b]  = sum_k wfT[k, h] * actT[k, b]      (stationary wfT tile)
    fg^T        = 0.5*zf^T + bfp[h]   (per-partition bias, scalar engine)
    fgx^T[h, b] = fg^T * hx^T[h, b]   (elementwise — NO PE transpose)
    ng^T, hy^T  analogous.
The host pre-transposes (and pre-casts to bf16) hx/input/w_f/w_n in
numpy, so the tensor engine runs ONLY the 512 real GEMM matmuls; the
~540 PE transposes of the previous version are gone entirely.

The only cross-core dependency is ng's contraction over the full hidden
dim of fgx: one AllGather of fgx^T per batch-half over each 4-core row
group, hidden under phase-1/phase-2 compute (phase 2 contracts its
input half first).
"""
import sys

sys.path.insert(0, "/opt/trn_rl_repo")

import numpy as np
import ml_dtypes
import concourse.bass as bass
import concourse.tile as tile
from concourse import bacc, mybir
from concourse.bass_utils import run_bass_kernel_spmd

F32 = mybir.dt.float32
BF16 = mybir.dt.bfloat16
IDENT = mybir.ActivationFunctionType.Identity
MULT = mybir.AluOpType.mult
ADD = mybir.AluOpType.add

B, H, I = 2048, 2048, 2048
R, C = 2, 4
BL = B // R            # 1024 batch rows per core
HC = H // C            # 512 output features per core
BB = BL // 2           # 512 batch cols per b-block
NKH = H // 128         # 16 k-tiles in the hx / fgx half
NKI = I // 128         # 16 k-tiles in the input half
NK = NKH + NKI         # 32 k-tiles total contraction
NA = HC // 128         # 4 hidden tiles per core slice
NWARM = 8              # HAM warm-up matmuls at kernel start

_NC_CACHE = None


def build():
    nc = bacc.Bacc(None, target_bir_lowering=False, debug=False)
    d_hxT = nc.dram_tensor("hxT", [H, BL], BF16, kind="ExternalInput").ap()
    d_inpT = nc.dram_tensor("inpT", [I, BL], BF16, kind="ExternalInput").ap()
    d_hxcT = nc.dram_tensor("hxcT", [HC, BL], BF16, kind="ExternalInput").ap()
    d_wfT = nc.dram_tensor("wfT", [H + I, HC], BF16, kind="ExternalInput").ap()
    d_wnT = nc.dram_tensor("wnT", [H + I, HC], BF16, kind="ExternalInput").ap()
    d_bf = nc.dram_tensor("bf", [128, NA], F32, kind="ExternalInput").ap()
    d_bn = nc.dram_tensor("bn", [128, NA], F32, kind="ExternalInput").ap()
    d_hyT = nc.dram_tensor("hyT", [HC, BL], F32, kind="ExternalOutput").ap()

    def kmaj(dram_ap, r0, nt, c0, ncols):
        """[nt*128, ncols] DRAM slab -> [128, nt, ncols] k-major AP."""
        return dram_ap[r0:r0 + nt * 128, c0:c0 + ncols].rearrange(
            "(t p) b -> p t b", p=128)

    with tile.TileContext(nc) as tc:
        with (
            tc.tile_pool(name="const", bufs=1) as const,
            tc.tile_pool(name="wf", bufs=1) as wfp,
            tc.tile_pool(name="wn", bufs=1) as wnp,
            tc.tile_pool(name="act", bufs=1) as actp,
            tc.tile_pool(name="gat", bufs=1) as gatp,
            tc.tile_pool(name="pers", bufs=1) as pers,
            tc.tile_pool(name="fgt", bufs=3) as fgtp,
            tc.tile_pool(name="scr", bufs=3) as scr,
            tc.tile_pool(name="outp", bufs=3) as outp,
            tc.tile_pool(name="dram", bufs=1, space="DRAM") as dram,
            tc.tile_pool(name="ps", bufs=6, space="PSUM") as ps,
            tc.tile_pool(name="pswarm", bufs=1, space="PSUM") as pswarm,
        ):
            # ---- HAM warm-up: keep the PE counted busy while DMAs land
            wm = const.tile([128, 512], BF16, tag="wm")
            nc.vector.memset(wm[:], 0.0009765625)
            psw = pswarm.tile([128, 512], F32, tag="warm")
            for i in range(NWARM):
                nc.tensor.matmul(psw[:], wm[:, :128], wm[:],
                                 start=(i == 0), stop=(i == NWARM - 1))

            # ---- persistent SBUF tensors
            s_wf = wfp.tile([128, NK, HC], BF16, tag="wf")
            s_wn = wnp.tile([128, NK, HC], BF16, tag="wn")
            s_hx = [actp.tile([128, NKH, BB], BF16, tag=f"hx{b}", name=f"s_hx{b}") for b in range(2)]
            s_inp = [actp.tile([128, NKI, BB], BF16, tag=f"in{b}", name=f"s_inp{b}") for b in range(2)]
            s_gat = [gatp.tile([128, NKH, BB], BF16, tag=f"gat{b}", name=f"s_gat{b}") for b in range(2)]
            s_hxc = pers.tile([128, NA, BL], BF16, tag="hxc")
            s_fgx = pers.tile([128, NA, BL], BF16, tag="fgx")
            s_omf = pers.tile([128, NA, BL], BF16, tag="omf")

            # DRAM bounce buffers for the two per-b-block AllGathers
            cc_in = [dram.tile([HC, BB], BF16, name=f"cc_in{b}") for b in range(2)]
            cc_out = [dram.tile([C, HC, BB], BF16, name=f"cc_out{b}") for b in range(2)]

            # ---- biases (gpsimd ring, tiny) + hxc slice
            bfr = const.tile([128, NA], F32, tag="bfr")
            bnr = const.tile([128, NA], F32, tag="bnr")
            nc.gpsimd.dma_start(bfr[:], d_bf[:])
            nc.gpsimd.dma_start(bnr[:], d_bn[:])
            nc.gpsimd.dma_start(s_hxc[:], kmaj(d_hxcT, 0, NA, 0, BL))
            bfp = const.tile([128, NA], F32, tag="bfp")
            bfm = const.tile([128, NA], F32, tag="bfm")
            nc.vector.tensor_scalar(bfp[:], bfr[:], 0.5, 0.5, MULT, ADD)
            nc.vector.tensor_scalar(bfm[:], bfr[:], -0.5, 0.5, MULT, ADD)

            # ---- bulk loads (sync HWDGE ring, FIFO = consumption order)
            # phase-1 b-block 0: wf k-groups interleaved with matching acts
            for g in range(4):
                nc.sync.dma_start(s_wf[:, g * 4:(g + 1) * 4, :],
                                  kmaj(d_wfT, g * 512, 4, 0, HC))
                nc.sync.dma_start(s_hx[0][:, g * 4:(g + 1) * 4, :],
                                  kmaj(d_hxT, g * 512, 4, 0, BB))
            for g in range(4):
                nc.sync.dma_start(s_wf[:, 16 + g * 4:16 + (g + 1) * 4, :],
                                  kmaj(d_wfT, 2048 + g * 512, 4, 0, HC))
                nc.sync.dma_start(s_inp[0][:, g * 4:(g + 1) * 4, :],
                                  kmaj(d_inpT, g * 512, 4, 0, BB))
            # phase-1 b-block 1 acts
            for g in range(4):
                nc.sync.dma_start(s_hx[1][:, g * 4:(g + 1) * 4, :],
                                  kmaj(d_hxT, g * 512, 4, BB, BB))
            for g in range(4):
                nc.sync.dma_start(s_inp[1][:, g * 4:(g + 1) * 4, :],
                                  kmaj(d_inpT, g * 512, 4, BB, BB))
            # w_n: input-half first (needed at phase-2 start), then hx-half
            for g in range(4):
                nc.sync.dma_start(s_wn[:, 16 + g * 4:16 + (g + 1) * 4, :],
                                  kmaj(d_wnT, 2048 + g * 512, 4, 0, HC))
            for g in range(4):
                nc.sync.dma_start(s_wn[:, g * 4:(g + 1) * 4, :],
                                  kmaj(d_wnT, g * 512, 4, 0, HC))

            # ---- phase 1: zf^T -> fg^T / (1-fg)^T / fgx^T, per (b-block, a)
            for bb in range(2):
                for a in range(NA):
                    acc = ps.tile([128, HC], F32, tag="acc")
                    for kt in range(NK):
                        rhs = (s_hx[bb][:, kt, :] if kt < NKH
                               else s_inp[bb][:, kt - NKH, :])
                        nc.tensor.matmul(acc[:],
                                         s_wf[:, kt, a * 128:(a + 1) * 128],
                                         rhs,
                                         start=(kt == 0), stop=(kt == NK - 1))
                    bcol = slice(bb * BB, (bb + 1) * BB)
                    fgt = fgtp.tile([128, BB], BF16, tag="fgt")
                    nc.scalar.activation(fgt[:], acc[:], IDENT,
                                         bias=bfp[:, a:a + 1], scale=0.5)
                    nc.scalar.activation(s_omf[:, a, bcol], acc[:], IDENT,
                                         bias=bfm[:, a:a + 1], scale=-0.5)
                    nc.vector.tensor_mul(s_fgx[:, a, bcol], fgt[:],
                                         s_hxc[:, a, bcol])
                    # stream this tile straight into the collective input
                    nc.scalar.dma_start(cc_in[bb][a * 128:(a + 1) * 128, :],
                                        s_fgx[:, a, bcol])
                nc.gpsimd.collective_compute(
                    "AllGather",
                    mybir.AluOpType.bypass,
                    replica_groups=[[0, 1, 2, 3], [4, 5, 6, 7]],
                    ins=[cc_in[bb].opt()],
                    outs=[cc_out[bb].opt()],
                )
                # read the gathered full-H fgx^T back (gpsimd ring)
                for q in range(4):
                    nc.gpsimd.dma_start(
                        s_gat[bb][:, q * 4:(q + 1) * 4, :],
                        cc_out[bb][q].rearrange("(t p) b -> p t b", p=128))

            # ---- phase 2: ng^T, hy^T. Input-half contraction first for all
            # four hidden tiles (CC-independent) so the AllGather hides.
            for bb in range(2):
                accs = []
                for a in range(NA):
                    acc = ps.tile([128, HC], F32, tag="acc")
                    accs.append(acc)
                    for i in range(NKI):
                        nc.tensor.matmul(acc[:],
                                         s_wn[:, NKH + i, a * 128:(a + 1) * 128],
                                         s_inp[bb][:, i, :],
                                         start=(i == 0), stop=False)
                for a in range(NA):
                    acc = accs[a]
                    for kt in range(NKH):
                        nc.tensor.matmul(acc[:],
                                         s_wn[:, kt, a * 128:(a + 1) * 128],
                                         s_gat[bb][:, kt, :],
                                         start=False, stop=(kt == NKH - 1))
                    bcol = slice(bb * BB, (bb + 1) * BB)
                    t = scr.tile([128, BB], F32, tag="t")
                    nc.vector.scalar_tensor_tensor(
                        t[:], acc[:], bnr[:, a:a + 1], s_omf[:, a, bcol],
                        ADD, MULT)
                    o = outp.tile([128, BB], F32, tag="o")
                    nc.vector.tensor_add(o[:], t[:], s_fgx[:, a, bcol])
                    nc.scalar.dma_start(
                        d_hyT[a * 128:(a + 1) * 128, bb * BB:(bb + 1) * BB],
                        o[:])

    nc.finalize()
    return nc


def _get_nc():
    global _NC_CACHE
    if _NC_CACHE is None:
        _NC_CACHE = build()
    return _NC_CACHE


def prepare_in_maps(input, hx, w_f, b_f, w_n, b_n):
    bf16 = ml_dtypes.bfloat16
    hxT_r, inpT_r = [], []
    for r in range(R):
        hxT_r.append(np.ascontiguousarray(
            hx[r * BL:(r + 1) * BL, :].T.astype(bf16)))
        inpT_r.append(np.ascontiguousarray(
            input[r * BL:(r + 1) * BL, :].T.astype(bf16)))
    wfT_c, wnT_c, bf_c, bn_c = [], [], [], []
    for c in range(C):
        wfT_c.append(np.ascontiguousarray(
            w_f[c * HC:(c + 1) * HC, :].T.astype(bf16)))
        wnT_c.append(np.ascontiguousarray(
            w_n[c * HC:(c + 1) * HC, :].T.astype(bf16)))
        bf_c.append(np.ascontiguousarray(
            b_f[c * HC:(c + 1) * HC].reshape(NA, 128).T.astype(np.float32)))
        bn_c.append(np.ascontiguousarray(
            b_n[c * HC:(c + 1) * HC].reshape(NA, 128).T.astype(np.float32)))
    in_maps = []
    for core in range(R * C):
        r, c = core // C, core % C
        in_maps.append({
            "hxT": hxT_r[r],
            "inpT": inpT_r[r],
            "hxcT": np.ascontiguousarray(hxT_r[r][c * HC:(c + 1) * HC, :]),
            "wfT": wfT_c[c],
            "wnT": wnT_c[c],
            "bf": bf_c[c],
            "bn": bn_c[c],
        })
    return in_maps


def assemble_output(results):
    rows = []
    for r in range(R):
        rows.append(np.concatenate(
            [np.asarray(results[r * C + c]["hyT"], dtype=np.float32).T
             for c in range(C)], axis=1))
    return np.ascontiguousarray(np.concatenate(rows, axis=0))


def kernel(input, hx, w_f, b_f, w_n, b_n, **_ignored):
    input = np.asarray(input, dtype=np.float32)
    hx = np.asarray(hx, dtype=np.float32)
    w_f = np.asarray(w_f, dtype=np.float32)
    b_f = np.asarray(b_f, dtype=np.float32)
    w_n = np.asarray(w_n, dtype=np.float32)
    b_n = np.asarray(b_n, dtype=np.float32)

    nc = _get_nc()
    in_maps = prepare_in_maps(input, hx, w_f, b_f, w_n, b_n)
    res = run_bass_kernel_spmd(nc, in_maps, list(range(R * C)))
    return assemble_output(res.results)


if __name__ == "__main__":
    rng = np.random.default_rng(0)
    inputs = {
        "input": rng.uniform(-1, 1, (B, I)).astype(np.float32),
        "hx": rng.uniform(-1, 1, (B, H)).astype(np.float32),
        "w_f": (rng.standard_normal((H, H + I)) / np.sqrt(H + I)).astype(np.float32),
        "b_f": (rng.standard_normal(H) / np.sqrt(H + I)).astype(np.float32),
        "w_n": (rng.standard_normal((H, H + I)) / np.sqrt(H + I)).astype(np.float32),
        "b_n": (rng.standard_normal(H) / np.sqrt(H + I)).astype(np.float32),
    }
    out = kernel(**inputs)
    x64 = {k: v.astype(np.float64) for k, v in inputs.items()}
    cat = np.concatenate([x64["hx"], x64["input"]], axis=1)
    fg = (cat @ x64["w_f"].T + x64["b_f"] + 1.0) * 0.5
    fgx = fg * x64["hx"]
    ng = np.concatenate([fgx, x64["input"]], axis=1) @ x64["w_n"].T + x64["b_n"]
    exp = (1.0 - fg) * ng + fgx
    err = np.abs(out - exp).max() / np.abs(exp).max()
    print("rel err:", err)
